# revision 16
# baseline (speedup 1.0000x reference)
"""GraphTransformerLayer (PyG TransformerConv style) on 8 trn2 NeuronCores.

Edges sorted by destination; nodes sharded 1/8 per core (each core owns all
edges into its node range -> no cross-core reduction, no collectives).
Per-edge tensors (x[src], edge_attr, one-hot dst masks in both orientations)
are laid out host-side in 128-edge tiles and streamed by direct DMA -- the
device does zero indirect gathers.  Per 128-edge tile:
  [kj|vj] = x_e @ [Wk|Wv] + attr_e @ [We|We]   (two PSUM-accumulated matmuls)
  q_e     = ohT @ q_block                       (one-hot matmul)
  logits  = rowsum_per_head(kj * q_e);  alpha = exp(logits/sqrt(C))
  acc    += oh^T @ [alpha*vj | alpha]           (scatter + denominators)
bf16 everywhere except PSUM accumulation / reductions / LayerNorm.  Node
epilogue (LN1 -> FFN -> LN2) runs in three SBUF-resident passes grouped by
activation-table set (Sqrt / Gelu / Sqrt).
"""
import numpy as np
import ml_dtypes

P = 128
H = 8
C = 16
GROUP = 4
N_CORES = 8

_BUILD_CACHE = {}

bf16_t = ml_dtypes.bfloat16


def _host_prep(x, edge_index, edge_attr):
    N, D = x.shape
    E = edge_index.shape[1]
    ED = edge_attr.shape[1]
    Nc = N // N_CORES
    NB = (Nc + P - 1) // P
    Npad = NB * P

    src = np.asarray(edge_index[0], dtype=np.int64)
    dst = np.asarray(edge_index[1], dtype=np.int64)
    order = np.argsort(dst, kind="stable")
    src_s = src[order].astype(np.int32)
    dst_s = dst[order].astype(np.int32)
    attr_s = np.asarray(edge_attr, dtype=np.float32)[order]

    bounds = np.empty((N_CORES, NB + 1), np.int64)
    for c in range(N_CORES):
        eb = np.searchsorted(dst_s, c * Nc + np.arange(NB + 1) * P)
        bounds[c] = np.minimum(eb, np.searchsorted(dst_s, (c + 1) * Nc))
    cnt = bounds[:, 1:] - bounds[:, :-1]
    Tb = np.maximum(1, np.ceil(cnt.max(axis=0) / P).astype(np.int64))
    off = np.concatenate([[0], np.cumsum(Tb)])
    Ttot = int(off[-1])

    x = np.asarray(x, dtype=np.float32)
    x_T_bf = np.ascontiguousarray(x.T).astype(bf16_t)  # [D, N]

    xgT_l, oh_l, ohT_l, attrT_l = [], [], [], []
    for c in range(N_CORES):
        srcslot = np.zeros(Ttot * P, np.int64)
        oh = np.zeros((P, Ttot, P), np.float32)
        ohT = np.zeros((P, Ttot, P), np.float32)
        attr_slots = np.zeros((Ttot * P, ED), np.float32)
        for b in range(NB):
            lo, hi = bounds[c, b], bounds[c, b + 1]
            ne = hi - lo
            o = int(off[b])
            pos = np.arange(ne)
            t_arr = o + pos // P
            p_arr = pos % P
            r_arr = dst_s[lo:hi] - c * Nc - b * P  # 0..127
            srcslot[t_arr * P + p_arr] = src_s[lo:hi]
            oh[p_arr, t_arr, r_arr] = 1.0
            ohT[r_arr, t_arr, p_arr] = 1.0
            attr_slots[o * P + pos] = attr_s[lo:hi]
        xgT_l.append(np.ascontiguousarray(x_T_bf[:, srcslot]))
        oh_l.append(oh.reshape(P, Ttot * P).astype(bf16_t))
        ohT_l.append(ohT.reshape(P, Ttot * P).astype(bf16_t))
        attrT_l.append(np.ascontiguousarray(attr_slots.T).astype(bf16_t))

    x_own_T_l, x_own_r_l = [], []
    for c in range(N_CORES):
        xo = np.zeros((Npad, D), np.float32)
        xo[:Nc] = x[c * Nc:(c + 1) * Nc]
        x_own_T_l.append(np.ascontiguousarray(xo.T).astype(bf16_t))
        x_own_r_l.append(np.ascontiguousarray(
            xo.reshape(NB, P, D).transpose(1, 0, 2)).reshape(P, NB * D)
            .astype(bf16_t))

    meta = dict(N=N, D=D, E=E, ED=ED, Nc=Nc, NB=NB, Npad=Npad,
                Tb=tuple(int(v) for v in Tb), Ttot=Ttot,
                off=tuple(int(v) for v in off))
    data = dict(xgT=xgT_l, oh=oh_l, ohT=ohT_l, attrT=attrT_l,
                x_own_T=x_own_T_l, x_own_r=x_own_r_l)
    return meta, data


def _build(meta):
    import concourse.bacc as bacc
    import concourse.bass as bass
    import concourse.tile as tile
    from concourse import mybir
    from concourse.masks import make_identity
    from contextlib import ExitStack

    f32 = mybir.dt.float32
    bf16 = mybir.dt.bfloat16
    Add = mybir.AluOpType.add
    Mult = mybir.AluOpType.mult

    N, D, ED = meta["N"], meta["D"], meta["ED"]
    NB, Npad = meta["NB"], meta["Npad"]
    Tb, off, Ttot = meta["Tb"], meta["off"], meta["Ttot"]

    nc = bacc.Bacc("TRN2", target_bir_lowering=False, debug=False,
                   num_devices=N_CORES)

    x_own_T = nc.dram_tensor("x_own_T", [D, Npad], bf16, kind="ExternalInput").ap()
    x_own_r = nc.dram_tensor("x_own_r", [P, NB * D], bf16, kind="ExternalInput").ap()
    xgT_d = nc.dram_tensor("xgT_d", [D, Ttot * P], bf16, kind="ExternalInput").ap()
    oh_d = nc.dram_tensor("oh_d", [P, Ttot * P], bf16, kind="ExternalInput").ap()
    ohT_d = nc.dram_tensor("ohT_d", [P, Ttot * P], bf16, kind="ExternalInput").ap()
    attrT = nc.dram_tensor("attrT", [ED, Ttot * P], bf16, kind="ExternalInput").ap()
    Wkv = nc.dram_tensor("Wkv", [D, 2 * D], bf16, kind="ExternalInput").ap()
    We2 = nc.dram_tensor("We2", [ED, 2 * D], bf16, kind="ExternalInput").ap()
    Wqs = nc.dram_tensor("Wqs", [D, 2 * D], bf16, kind="ExternalInput").ap()
    Wf1 = nc.dram_tensor("Wf1", [D, 4 * D], bf16, kind="ExternalInput").ap()
    Wf2 = nc.dram_tensor("Wf2", [4 * D, D], bf16, kind="ExternalInput").ap()
    out = nc.dram_tensor("out", [Npad, D], f32, kind="ExternalOutput").ap()

    def ap_append(ap, n):
        a = ap.copy()
        a.ap = a.ap + [[0, n]]
        return a

    ctx = ExitStack()
    with tile.TileContext(nc) as tc:
        const = ctx.enter_context(tc.tile_pool(name="const", bufs=1))
        Wkv_sb = const.tile([D, 2 * D], bf16)
        nc.sync.dma_start(out=Wkv_sb[:], in_=Wkv[:, :])
        We2_sb = const.tile([ED, 2 * D], bf16)
        nc.sync.dma_start(out=We2_sb[:], in_=We2[:, :])
        Wqs_sb = const.tile([D, 2 * D], bf16)
        nc.sync.dma_start(out=Wqs_sb[:], in_=Wqs[:, :])
        Wf1_sb = const.tile([D, 4 * D], bf16)
        nc.sync.dma_start(out=Wf1_sb[:], in_=Wf1[:, :])
        Wf2_sb = const.tile([D, 4, D], bf16)
        for j in range(4):
            nc.sync.dma_start(out=Wf2_sb[:, j, :], in_=Wf2[j * D:(j + 1) * D, :])
        ident = const.tile([P, P], bf16)
        make_identity(nc, ident[:])
        eps_t = const.tile([P, 1], f32)
        nc.vector.memset(eps_t[:], 1e-5)
        xoT_sb = const.tile([D, Npad], bf16)
        nc.sync.dma_start(out=xoT_sb[:], in_=x_own_T[:, :])
        xor_sb = const.tile([P, NB * D], bf16)
        nc.sync.dma_start(out=xor_sb[:], in_=x_own_r[:, :])
        qsk_sb = const.tile([P, NB, 2 * D], bf16)
        conv_all = const.tile([P, NB * D], f32)
        h_all = const.tile([P, NB * D], f32)

        # ---- phase B: q + skip per own block, kept in SBUF ----
        with tc.tile_pool(name="pb_ps", bufs=2, space="PSUM") as pb_ps:
            for b in range(NB):
                pB = pb_ps.tile([P, 2 * D], f32, tag="pb")
                nc.tensor.matmul(pB[:], lhsT=xoT_sb[:, b * P:(b + 1) * P],
                                 rhs=Wqs_sb[:], start=True, stop=True)
                nc.scalar.copy(out=qsk_sb[:, b, :], in_=pB[:])

        # ---- phase C: edge aggregation per block ----
        with tc.tile_pool(name="pc_gi", bufs=4) as pc_gi, \
             tc.tile_pool(name="pc_w", bufs=4) as pc_w, \
             tc.tile_pool(name="pc_kv", bufs=2, space="PSUM") as pc_kv, \
             tc.tile_pool(name="pc_qps", bufs=2, space="PSUM") as pc_qps, \
             tc.tile_pool(name="pc_acc", bufs=2, space="PSUM") as pc_acc, \
             tc.tile_pool(name="pc_ep", bufs=2) as pc_ep:
            for b in range(NB):
                T, o = Tb[b], off[b]
                acc = pc_acc.tile([P, D + H], f32, tag="acc")
                done = 0
                while done < T:
                    G = min(GROUP, T - done)
                    og = (o + done) * P
                    xg_sb = pc_gi.tile([D, GROUP * P], bf16, tag="xg")
                    nc.sync.dma_start(out=xg_sb[:, :G * P],
                                      in_=xgT_d[:, og:og + G * P])
                    at_sb = pc_gi.tile([ED, GROUP * P], bf16, tag="at")
                    nc.sync.dma_start(out=at_sb[:, :G * P],
                                      in_=attrT[:, og:og + G * P])
                    oh_sb = pc_gi.tile([P, GROUP, P], bf16, tag="oh")
                    nc.scalar.dma_start(
                        out=oh_sb[:, :G, :].rearrange("p t e -> p (t e)"),
                        in_=oh_d[:, og:og + G * P])
                    ohT_sb = pc_gi.tile([P, GROUP, P], bf16, tag="ohT")
                    nc.scalar.dma_start(
                        out=ohT_sb[:, :G, :].rearrange("p t e -> p (t e)"),
                        in_=ohT_d[:, og:og + G * P])
                    kv_ps = pc_kv.tile([P, GROUP, 2 * D], f32, tag="kvps")
                    qe_ps = pc_qps.tile([P, GROUP, D], f32, tag="qeps")
                    for j in range(G):
                        nc.tensor.matmul(kv_ps[:, j, :],
                                         lhsT=xg_sb[:, j * P:(j + 1) * P],
                                         rhs=Wkv_sb[:], start=True, stop=False)
                        nc.tensor.matmul(kv_ps[:, j, :],
                                         lhsT=at_sb[:, j * P:(j + 1) * P],
                                         rhs=We2_sb[:], start=False, stop=True)
                        nc.tensor.matmul(qe_ps[:, j, :], lhsT=ohT_sb[:, j, :],
                                         rhs=qsk_sb[:, b, 0:D],
                                         start=True, stop=True)
                    kj_sb = pc_w.tile([P, GROUP, D], bf16, tag="kj")
                    nc.scalar.copy(out=kj_sb[:, :G, :], in_=kv_ps[:, :G, 0:D])
                    prod = pc_w.tile([P, GROUP, D], bf16, tag="prod")
                    nc.vector.tensor_tensor(
                        out=prod[:, :G, :], in0=kj_sb[:, :G, :],
                        in1=qe_ps[:, :G, :], op=Mult)
                    logit = pc_w.tile([P, GROUP * H], f32, tag="logit")
                    nc.vector.tensor_reduce(
                        out=logit[:, :G * H],
                        in_=prod[:, :G, :].rearrange(
                            "p t (h c) -> p (t h) c", h=H),
                        axis=mybir.AxisListType.X, op=Add)
                    expc = pc_w.tile([P, GROUP * H], f32, tag="expc")
                    nc.scalar.activation(out=expc[:, :G * H], in_=logit[:, :G * H],
                                         func=mybir.ActivationFunctionType.Exp,
                                         scale=1.0 / np.sqrt(C))
                    rhs_st = pc_w.tile([P, GROUP, D + H], bf16, tag="rhs")
                    nc.vector.tensor_copy(
                        out=rhs_st[:, :G, D:D + H],
                        in_=expc[:, :G * H].rearrange("p (t h) -> p t h", h=H))
                    nc.vector.tensor_tensor(
                        out=rhs_st[:, :G, 0:D].rearrange(
                            "p t (h c) -> p t h c", h=H),
                        in0=kv_ps[:, :G, D:2 * D].rearrange(
                            "p t (h c) -> p t h c", h=H),
                        in1=ap_append(expc[:, :G * H].rearrange(
                            "p (t h) -> p t h", h=H), C),
                        op=Mult)
                    for j in range(G):
                        t = done + j
                        nc.tensor.matmul(acc[:, :], lhsT=oh_sb[:, j, :],
                                         rhs=rhs_st[:, j, :],
                                         start=(t == 0), stop=(t == T - 1))
                    done += G

                # block epilogue: conv = agg/denom + skip + x
                dn = pc_ep.tile([P, H], f32, tag="dn")
                nc.vector.tensor_scalar_max(out=dn[:], in0=acc[:, D:D + H],
                                            scalar1=1e-30)
                rec = pc_ep.tile([P, H], f32, tag="rec")
                nc.vector.reciprocal(out=rec[:], in_=dn[:])
                cv = conv_all[:, b * D:(b + 1) * D]
                nc.vector.tensor_tensor(
                    out=cv.rearrange("p (h c) -> p h c", h=H),
                    in0=acc[:, 0:D].rearrange("p (h c) -> p h c", h=H),
                    in1=ap_append(rec[:], C), op=Mult)
                nc.vector.tensor_tensor(out=cv, in0=cv,
                                        in1=qsk_sb[:, b, D:2 * D], op=Add)
                nc.vector.tensor_tensor(
                    out=cv, in0=cv, in1=xor_sb[:, b * D:(b + 1) * D], op=Add)

        # ---- phase D: LN1 -> FFN -> LN2, rsqrt via Newton on DVE ----
        i32 = mybir.dt.int32
        MAGIC = 0x5f3759df

        def batched_ln(pool, src_all, dst_writer):
            """LayerNorm all NB blocks of src_all; dst_writer(b, ts_kwargs)"""
            mean_all = pool.tile([P, NB], f32, tag="mean")
            var_all = pool.tile([P, NB], f32, tag="var")
            for b in range(NB):
                st = pool.tile([P, 6], f32, tag="st")
                nc.vector.bn_stats(out=st[:], in_=src_all[:, b * D:(b + 1) * D])
                mv = pool.tile([P, 2], f32, tag="mv")
                nc.vector.bn_aggr(out=mv[:], in_=st[:])
                nc.vector.tensor_copy(out=mean_all[:, b:b + 1], in_=mv[:, 0:1])
                nc.vector.tensor_copy(out=var_all[:, b:b + 1], in_=mv[:, 1:2])
            # rstd = rsqrt(var + eps), Quake seed + 3 Newton iterations
            vv = pool.tile([P, NB], f32, tag="vv")
            nc.vector.tensor_scalar(out=vv[:], in0=var_all[:], scalar1=1e-5,
                                    scalar2=None, op0=Add)
            sh = pool.tile([P, NB], i32, tag="sh")
            nc.vector.tensor_scalar(
                out=sh[:], in0=vv[:].bitcast(i32), scalar1=1,
                scalar2=None, op0=mybir.AluOpType.logical_shift_right)
            magic_t = pool.tile([P, NB], i32, tag="magic")
            nc.vector.memset(magic_t[:], MAGIC)
            y = pool.tile([P, NB], f32, tag="y")
            nc.vector.tensor_tensor(
                out=y[:].bitcast(i32), in0=magic_t[:], in1=sh[:],
                op=mybir.AluOpType.subtract)
            t1 = pool.tile([P, NB], f32, tag="t1")
            for _ in range(3):
                nc.vector.tensor_tensor(out=t1[:], in0=y[:], in1=y[:], op=Mult)
                nc.vector.tensor_tensor(out=t1[:], in0=t1[:], in1=vv[:], op=Mult)
                nc.vector.tensor_scalar(out=t1[:], in0=t1[:], scalar1=-0.5,
                                        scalar2=1.5, op0=Mult, op1=Add)
                nc.vector.tensor_tensor(out=y[:], in0=y[:], in1=t1[:], op=Mult)
            for b in range(NB):
                dst_writer(b, dict(scalar1=mean_all[:, b:b + 1],
                                   scalar2=y[:, b:b + 1],
                                   op0=mybir.AluOpType.subtract, op1=Mult))

        with tc.tile_pool(name="pd1", bufs=2) as pd1:
            def w1(b, kw):
                nc.vector.tensor_scalar(
                    out=h_all[:, b * D:(b + 1) * D],
                    in0=conv_all[:, b * D:(b + 1) * D], **kw)
            batched_ln(pd1, conv_all, w1)

        # ---- FFN for all blocks (gelu table); h2 reuses conv_all ----
        with tc.tile_pool(name="pd2", bufs=3) as pd2, \
             tc.tile_pool(name="pd2_ps", bufs=2, space="PSUM") as pd2_ps:
            for b in range(NB):
                hs = h_all[:, b * D:(b + 1) * D]
                hb = pd2.tile([P, D], bf16, tag="hb")
                nc.vector.tensor_copy(out=hb[:], in_=hs)
                tr_ps = pd2_ps.tile([P, D], bf16, tag="trps")
                nc.tensor.transpose(out=tr_ps[:], in_=hb[:], identity=ident[:])
                h1T = pd2.tile([P, D], bf16, tag="h1T")
                nc.vector.tensor_copy(out=h1T[:], in_=tr_ps[:])
                o2_ps = pd2_ps.tile([P, D], f32, tag="o2ps")
                m1 = pd2_ps.tile([P, 4, D], f32, tag="m1ps")
                for j in range(4):
                    nc.tensor.matmul(m1[:, j, :], lhsT=Wf1_sb[:, j * D:(j + 1) * D],
                                     rhs=h1T[:], start=True, stop=True)
                gj = pd2.tile([P, 4, D], bf16, tag="gj")
                nc.scalar.activation(out=gj[:], in_=m1[:],
                                     func=mybir.ActivationFunctionType.Gelu)
                for j in range(4):
                    nc.tensor.matmul(o2_ps[:], lhsT=gj[:, j, :], rhs=Wf2_sb[:, j, :],
                                     start=(j == 0), stop=(j == 3))
                nc.vector.tensor_tensor(
                    out=conv_all[:, b * D:(b + 1) * D], in0=hs, in1=o2_ps[:],
                    op=Add)

        # ---- LN2 for all blocks + output ----
        with tc.tile_pool(name="pd3", bufs=2) as pd3, \
             tc.tile_pool(name="pd3o", bufs=3) as pd3o:
            def w3(b, kw):
                ot = pd3o.tile([P, D], f32, tag="ot")
                nc.vector.tensor_scalar(
                    out=ot[:], in0=conv_all[:, b * D:(b + 1) * D], **kw)
                nc.sync.dma_start(out=out[b * P:(b + 1) * P, :], in_=ot[:])
            batched_ln(pd3, conv_all, w3)

        ctx.close()

    nc.compile()
    return nc


def kernel(**inputs):
    import os
    from concourse.bass_utils import run_bass_kernel_spmd

    x = np.asarray(inputs["x"], dtype=np.float32)
    meta, data = _host_prep(x, inputs["edge_index"], inputs["edge_attr"])

    # biases are zero and LN affine params are identity in this problem;
    # the kernel skips them, so verify that assumption on the real inputs
    for k in ("bq", "bk", "bv", "bskip", "bf1", "bf2", "b1", "b2"):
        assert not np.any(np.asarray(inputs[k])), f"nonzero bias {k}"
    assert np.all(np.asarray(inputs["g1"]) == 1.0)
    assert np.all(np.asarray(inputs["g2"]) == 1.0)

    key = (meta["N"], meta["D"], meta["ED"], meta["Tb"])
    if key not in _BUILD_CACHE:
        _BUILD_CACHE[key] = _build(meta)
    nc = _BUILD_CACHE[key]

    tobf = lambda a: np.ascontiguousarray(np.asarray(a, np.float32)).astype(bf16_t)
    We = np.asarray(inputs["We"], np.float32)
    common = dict(
        Wkv=tobf(np.concatenate([np.asarray(inputs["Wk"], np.float32),
                                 np.asarray(inputs["Wv"], np.float32)], axis=1)),
        We2=tobf(np.concatenate([We, We], axis=1)),
        Wqs=tobf(np.concatenate([np.asarray(inputs["Wq"], np.float32),
                                 np.asarray(inputs["Wskip"], np.float32)], axis=1)),
        Wf1=tobf(inputs["Wf1"]), Wf2=tobf(inputs["Wf2"]))
    in_maps = []
    for c in range(N_CORES):
        m = dict(common)
        m["x_own_T"] = data["x_own_T"][c]
        m["x_own_r"] = data["x_own_r"][c]
        m["xgT_d"] = data["xgT"][c]
        m["oh_d"] = data["oh"][c]
        m["ohT_d"] = data["ohT"][c]
        m["attrT"] = data["attrT"][c]
        in_maps.append(m)

    trace_cores = os.environ.get("KERNEL_TRACE_CORES")
    kwargs = {}
    if trace_cores:
        kwargs["trace"] = True
        kwargs["trace_cores"] = [int(c) for c in trace_cores.split(",")]
    res = run_bass_kernel_spmd(nc, in_maps, list(range(N_CORES)), **kwargs)
    globals()["LAST_RESULTS"] = res
    Nc = meta["Nc"]
    outp = np.concatenate([res.results[c]["out"][:Nc] for c in range(N_CORES)],
                          axis=0)
    return outp.astype(np.float32)
